# revision 1
# baseline (speedup 1.0000x reference)
"""Bistable recurrent cell layer on 8 Trainium2 NeuronCores.

Data-parallel over batch: each core owns B/8 = 8 batch rows, computes the
three input projections (x@kr, x@kz, x@kh) on the tensor engine, then runs
the T=512 sequential scan on DVE/ACT/GPSIMD, all in one NEFF.

Key tricks:
- Host pre-scales kz, bz, mz by 1/2 so z = sigmoid(xz + h*mz) becomes
  (tanh(sz')+1)/2 with sz' = xz' + h*mz' — every activation in the scan is
  a tanh, so the two first-stage activations fuse into one ACT instruction.
- The running state is stored as adjacent [h | h/2] column pairs so the
  fused wide-add reads both without broadcast APs.
- The scan runs as two independent batch-groups (b 0:4 / 4:8) with fully
  separate tiles, interleaved, to hide the per-step cross-engine latency.
- GEMMs run in bf16 with a 3-term error-compensated split
  (x_hi*k_hi + x_hi*k_lo + x_lo*k_hi) accumulated in fp32 PSUM: bf16 speed,
  ~1e-5 relative accuracy.
- Host pre-transposes x to [D, B_loc*T] per core (the GEMM contracts over
  d on partitions) and re-transposes outputs.
"""
import os
import sys

for _p in ('/opt/trn_rl_repo', os.path.dirname(os.path.abspath(__file__))):
    if _p not in sys.path:
        sys.path.insert(0, _p)

import numpy as np
import ml_dtypes
from contextlib import ExitStack

import concourse.bass as bass
import concourse.tile as tile
from concourse.tile import add_dep_helper
from concourse import bacc, mybir
from concourse.bass_utils import run_bass_kernel_spmd

F32 = mybir.dt.float32
F32R = mybir.dt.float32r
BF16 = mybir.dt.bfloat16
AF = mybir.ActivationFunctionType
OP = mybir.AluOpType

B, T, D, H = 64, 512, 512, 512
NCORES = 8
BL = B // NCORES

last_exec_time_ns = None


def build_body(ctx, tc, aps, cfg):
    nc = tc.nc
    Tt, TC, Bl = cfg['T'], cfg['TC'], cfg['BL']
    nchunk = Tt // TC
    ngrp = cfg['ngrp']
    bg = Bl // ngrp
    gemm = cfg['gemm']          # 'bf16x3' | 'f32r' | 'f32'
    use_gps = cfg.get('use_gps', True)

    weights = ctx.enter_context(tc.tile_pool(name='weights', bufs=1))
    xt_pool = ctx.enter_context(tc.tile_pool(name='xt', bufs=2))
    prod_pool = ctx.enter_context(tc.tile_pool(name='prod', bufs=2))
    ys_pool = ctx.enter_context(tc.tile_pool(name='ys', bufs=2))
    state = ctx.enter_context(tc.tile_pool(name='state', bufs=1))
    tmp = ctx.enter_context(tc.tile_pool(name='tmp', bufs=3))
    psum_pool = ctx.enter_context(tc.tile_pool(name='psum', bufs=2, space='PSUM'))
    spsum = ctx.enter_context(tc.tile_pool(name='spsum', bufs=2, space='PSUM'))

    dt_mm = {'bf16x3': BF16, 'f32r': F32R, 'f32': F32}[gemm]
    kparts = ('h', 'l') if gemm == 'bf16x3' else ('',)

    # ---- weights: k order 0=r, 1=z(pre-halved), 2=h ----
    k_sb = {}
    for name in ('kr', 'kz', 'kh'):
        for part in kparts:
            t = weights.tile([128, 4, H], dt_mm, tag=name + part)
            nc.sync.dma_start(
                t[:], aps[name + part].rearrange('(dc p) h -> p dc h', p=128))
            k_sb[name + part] = t
    knames = ('kr', 'kz', 'kh')

    if cfg['general_bias']:
        b_sb = weights.tile([128, 2, 4], F32, tag='bias')  # [p, (r,z'), hb]
        nc.sync.dma_start(b_sb[:, 0, :], aps['br'].rearrange('(hb p) -> p hb', p=128))
        nc.sync.dma_start(b_sb[:, 1, :], aps['bz'].rearrange('(hb p) -> p hb', p=128))
    if cfg['general_m']:
        # [p, (mr, mz), hb, b] — z column multiplies the h/2 pair entry
        m_sb = weights.tile([128, 2, 4, Bl], F32, tag='m')
        for i, nm in enumerate(('mr', 'mz')):
            src = aps[nm].rearrange('(hb p) -> p hb', p=128).unsqueeze(2)
            nc.sync.dma_start(m_sb[:, i, :, :], src.broadcast_to([128, 4, Bl]))

    halfc = weights.tile([128, 4, bg], F32, tag='halfc')
    nc.vector.memset(halfc[:], 0.5)

    # state h: [p, hb, b]
    hl = state.tile([128, 4, Bl], F32, tag='h_last0')
    h_last = [hl]
    if cfg['general_h0']:
        h0_src = aps['h0'].rearrange('b (hb p) -> p hb b', p=128)
        for hb in range(4):
            nc.sync.dma_start(hl[:, hb], h0_src[:, hb])
    else:
        nc.vector.memset(hl[:], 0.0)

    xt_src = {p: aps['xt' + p].rearrange('(dc p) (b t) -> p dc b t', p=128, b=Bl)
              for p in kparts}
    yt_dst = aps['yt'].rearrange('(hb p) (b t) -> p hb b t', p=128, b=Bl)

    for ci in range(nchunk):
        t0, t1_ = ci * TC, (ci + 1) * TC

        xt_t = {}
        for part in kparts:
            xt = xt_pool.tile([128, 4, Bl, TC], dt_mm, tag='xt' + part)
            for dc in range(4):
                nc.sync.dma_start(xt[:, dc], xt_src[part][:, dc, :, t0:t1_])
            xt_t[part] = xt

        # prod [p, k(r,z',h), hb, b, t]
        prod = prod_pool.tile([128, 3, 4, Bl, TC], F32, tag='prod')
        icopy = 0
        for ht in range(4):
            for kj, kn in enumerate(knames):
                ps = psum_pool.tile([128, Bl * TC], F32, tag='ps')
                if gemm == 'bf16x3':
                    terms = [('h', 'h'), ('h', 'l'), ('l', 'h')]
                else:
                    terms = [('', '')]
                nmm = 4 * len(terms)
                imm = 0
                for dc in range(4):
                    for kp, xp in terms:
                        nc.tensor.matmul(
                            ps[:], k_sb[kn + kp][:, dc, ht * 128:(ht + 1) * 128],
                            xt_t[xp][:, dc, :, :],
                            start=(imm == 0), stop=(imm == nmm - 1))
                        imm += 1
                dest = prod[:, kj, ht, :, :]
                ps_v = ps[:].rearrange('p (b t) -> p b t', b=Bl)
                if cfg['general_bias'] and kj < 2:
                    nc.scalar.activation(
                        dest, ps_v, AF.Identity, bias=b_sb[:, kj, ht:ht + 1])
                else:
                    nc.scalar.copy(dest, ps_v)
                    icopy += 1

        # ---- scan over this chunk (chain-latency-minimal form) ----
        # Per-step critical chain (one group, FD32):
        #   s = A_t + h -> t1 = tanh(s) -> m1 = t1*h -> cc = m1 + P
        #   -> g = tanh(cc) -> m = u*g -> h' = m + m2
        # Everything else runs off-chain in parallel:
        #   GPS: P = h + C_t, sz = B_t + h, m2 = z*h
        #   ACT: tz = tanh(0.5*sz)   DVE-ts: z = 0.5+tz/2, u = 0.5-tz/2
        # where z = sigmoid(xz + h) and h' = z*h + (1-z)*g.
        ys = ys_pool.tile([128, 4, Bl, TC], F32, tag='ys', name=f'ys_{ci}')
        eng = nc.gpsimd if use_gps else nc.vector
        for tt in range(TC):
            h = h_last[0][:] if tt == 0 else ys[:, :, :, tt - 1]
            At = prod[:, 0, :, :, tt]
            Bt = prod[:, 1, :, :, tt]
            Ct = prod[:, 2, :, :, tt]
            sh = [128, 4, Bl]
            nm = f'_{ci}_{tt}'

            if cfg['general_m']:
                # r-branch: s = A_t + h*mr ; z-branch arg: (B_t + h*mz)/2
                hmr = tmp.tile(sh, F32, tag='hmr', name='hmr' + nm)
                nc.vector.tensor_mul(hmr[:], h, m_sb[:, 0])
                hmz = tmp.tile(sh, F32, tag='hmz', name='hmz' + nm)
                eng.tensor_mul(hmz[:], h, m_sb[:, 1])
                s_in, z_in = hmr[:], hmz[:]
            else:
                s_in, z_in = h, h

            ss = spsum.tile(sh, F32, tag='ss', name='ss' + nm, bufs=1)
            nc.vector.tensor_add(ss[:], s_in, At)
            sz = spsum.tile(sh, F32, tag='sz', name='sz' + nm, bufs=1)
            nc.vector.tensor_add(sz[:], z_in, Bt)
            PP = tmp.tile(sh, F32, tag='PP', name='PP' + nm)
            nc.vector.tensor_add(PP[:], h, Ct)

            t1 = spsum.tile(sh, F32, tag='t1', name='t1' + nm, bufs=1)
            i_t1 = nc.scalar.activation(t1[:], ss[:], AF.Tanh)
            tz = tmp.tile(sh, F32, tag='tz', name='tz' + nm)
            i_tz = nc.scalar.activation(tz[:], sz[:], AF.Tanh, scale=0.5)
            add_dep_helper(i_tz.ins, i_t1.ins, sync=False,
                           reason='tz waits for t1 on ACT')

            m1 = tmp.tile(sh, F32, tag='m1', name='m1' + nm)
            nc.vector.tensor_mul(m1[:], t1[:], h)
            cc = spsum.tile(sh, F32, tag='cc', name='cc' + nm, bufs=1)
            i_cc = nc.vector.tensor_add(cc[:], m1[:], PP[:])
            gg = spsum.tile(sh, F32, tag='gg', name='gg' + nm, bufs=1)
            nc.scalar.activation(gg[:], cc[:], AF.Tanh)

            zz = tmp.tile(sh, F32, tag='zz', name='zz' + nm)
            i_zz = nc.vector.tensor_scalar(zz[:], tz[:], 0.5, 0.5, OP.mult, OP.add)
            add_dep_helper(i_zz.ins, i_cc.ins, sync=False,
                           reason='keep cc ahead of zz on DVE')
            uu = tmp.tile(sh, F32, tag='uu', name='uu' + nm)
            nc.vector.tensor_scalar(uu[:], tz[:], -0.5, 0.5, OP.mult, OP.add)
            m2 = tmp.tile(sh, F32, tag='m2', name='m2' + nm)
            nc.vector.tensor_mul(m2[:], zz[:], h)

            mm = tmp.tile(sh, F32, tag='mm', name='mm' + nm)
            nc.vector.tensor_mul(mm[:], uu[:], gg[:])
            nc.vector.tensor_add(ys[:, :, :, tt], mm[:], m2[:])

        nc.gpsimd.tensor_copy(h_last[0][:], ys[:, :, :, TC - 1])
        for hb in range(4):
            nc.sync.dma_start(yt_dst[:, hb, :, t0:t1_], ys[:, hb])


def build_program(cfg):
    nc = bacc.Bacc('TRN2', target_bir_lowering=False, debug=False)
    Tt, Bl = cfg['T'], cfg['BL']
    gemm = cfg['gemm']
    aps = {}
    dt_mm = {'bf16x3': BF16, 'f32r': F32R, 'f32': F32}[gemm]
    kparts = ('h', 'l') if gemm == 'bf16x3' else ('',)
    for part in kparts:
        aps['xt' + part] = nc.dram_tensor(
            'xt' + part, [D, Bl * Tt], dt_mm, kind='ExternalInput').ap()
        for name in ('kr', 'kz', 'kh'):
            aps[name + part] = nc.dram_tensor(
                name + part, [D, H], dt_mm, kind='ExternalInput').ap()
    if cfg['general_m']:
        for name in ('mr', 'mz'):
            aps[name] = nc.dram_tensor(name, [H], F32, kind='ExternalInput').ap()
    if cfg['general_bias']:
        for name in ('br', 'bz'):
            aps[name] = nc.dram_tensor(name, [H], F32, kind='ExternalInput').ap()
    if cfg['general_h0']:
        aps['h0'] = nc.dram_tensor('h0', [Bl, H], F32, kind='ExternalInput').ap()
    aps['yt'] = nc.dram_tensor('yt', [H, Bl * Tt], F32, kind='ExternalOutput').ap()

    with tile.TileContext(nc) as tc, ExitStack() as ctx:
        build_body(ctx, tc, aps, cfg)
    nc.compile()
    return nc


def _install_trace_hook():
    """Register the NTFF profile hook this image's antenv lacks, and neuter
    the cloud artifact upload, so trace=True works locally."""
    import types
    if 'antenv.axon_hooks' not in sys.modules:
        import antenv
        mod = types.ModuleType('antenv.axon_hooks')
        state = {'hook': None}
        mod.set_axon_ntff_profile_hook = lambda h: state.__setitem__('hook', h)
        mod.get_axon_ntff_profile_hook = lambda: state['hook']
        sys.modules['antenv.axon_hooks'] = mod
        antenv.axon_hooks = mod
        from trn_agent_boot.trn_boot import _ntff_profile_via_ctypes
        mod.set_axon_ntff_profile_hook(
            _ntff_profile_via_ctypes('/opt/axon/libaxon_pjrt.so'))
    import concourse.bass_utils as bu
    bu.upload_artifacts = lambda tmpdir: f"local:{tmpdir}"


_programs = {}


def _get_program(key, cfg):
    if key not in _programs:
        _programs[key] = build_program(cfg)
    return _programs[key]


def _bf16_split(a):
    hi = a.astype(ml_dtypes.bfloat16)
    lo = (a - hi.astype(np.float32)).astype(ml_dtypes.bfloat16)
    return hi, lo


def kernel(x, h0, kz, kr, kh, mz, mr, bz, br):
    global last_exec_time_ns
    x = np.asarray(x, dtype=np.float32)
    h0 = np.asarray(h0, dtype=np.float32)
    kz, kr, kh = (np.asarray(a, dtype=np.float32) for a in (kz, kr, kh))
    mz, mr, bz, br = (np.asarray(a, dtype=np.float32) for a in (mz, mr, bz, br))

    cfg = {
        'T': T, 'TC': int(os.environ.get('BRC_TC', '64')), 'BL': BL,
        'ngrp': int(os.environ.get('BRC_NGRP', '2')),
        'general_m': not (np.all(mz == 1.0) and np.all(mr == 1.0)),
        'general_bias': not (np.all(bz == 0.0) and np.all(br == 0.0)),
        'general_h0': not np.all(h0 == 0.0),
        'gemm': os.environ.get('BRC_GEMM', 'f32'),
        'use_gps': os.environ.get('BRC_NOGPS', '0') != '1',
    }
    key = tuple(sorted(cfg.items()))
    nc = _get_program(key, cfg)
    gemm = cfg['gemm']

    ks = {'kr': kr, 'kz': kz, 'kh': kh}
    kmaps = {}
    if gemm == 'bf16x3':
        for name, a in ks.items():
            kmaps[name + 'h'], kmaps[name + 'l'] = _bf16_split(a)
    else:
        kmaps = ks

    in_maps = []
    for c in range(NCORES):
        xi = x[c * BL:(c + 1) * BL]                      # [BL, T, D]
        xt = np.ascontiguousarray(
            xi.transpose(2, 0, 1).reshape(D, BL * T))     # [D, BL*T]
        m = dict(kmaps)
        if gemm == 'bf16x3':
            m['xth'], m['xtl'] = _bf16_split(xt)
        else:
            m['xt'] = xt
        if cfg['general_m']:
            # z column multiplies the h/2 pair entry, so mz stays unscaled
            m['mr'] = mr
            m['mz'] = mz
        if cfg['general_bias']:
            m['br'] = br
            m['bz'] = bz
        if cfg['general_h0']:
            m['h0'] = np.ascontiguousarray(h0[c * BL:(c + 1) * BL])
        in_maps.append(m)

    trace = os.environ.get('BRC_TRACE', '0') == '1'
    if trace:
        _install_trace_hook()
    res = run_bass_kernel_spmd(
        nc, in_maps, core_ids=list(range(NCORES)), trace=trace)
    last_exec_time_ns = res.exec_time_ns
    kernel.last_results = res

    out = np.empty((B, T, H), dtype=np.float32)
    for c in range(NCORES):
        yt = res.results[c]['yt']                         # [H, BL*T]
        out[c * BL:(c + 1) * BL] = (
            yt.reshape(H, BL, T).transpose(1, 2, 0))      # [BL, T, H]
    return out

